# revision 5
# baseline (speedup 1.0000x reference)
"""Trainium2 Bass kernel for nn_DFDgraph (gnn_message_passing).

Pipeline per batch element (one NeuronCore each, 8 total):
  x (2048, 288) --rfft-mag--> (2048, 145) --minmax+l2--> xn
  h = LN(relu(cat[xn @ Wd0, te_norm] @ We0))            (2048, 64)
  adj = relu((h * w) @ h^T)                             (2048, 2048)
  out = top10_row_mask(adj) / (rowsum_kept + 1e-5)

The rfft is computed as two matmuls against host-precomputed DFT
cos/sin matrices (288 x 145, ortho-normalized). Top-10 per row uses the
DVE max8 / match_replace8 instructions: max8 -> zap top-8 -> max8 gives
the exact 10 largest values per row; kept = (adj >= v10) * adj in one
fused scalar_tensor_tensor pass, scaled by 1/(sum_top10 + 1e-5) on ACT.
"""

import numpy as np
from contextlib import ExitStack

import concourse.bass as bass
import concourse.mybir as mybir
from concourse import bacc
from concourse import tile
from concourse import masks
from concourse.bass_utils import run_bass_kernel_spmd

F32 = mybir.dt.float32
AX = mybir.AxisListType
OP = mybir.AluOpType
AF = mybir.ActivationFunctionType

B, N, T, H, EMB, TOPK = 8, 2048, 288, 64, 24, 10
F = T // 2 + 1          # 145
P = 128                 # rows per tile
NT = N // P             # 16 tiles
KC = 96                 # DFT contraction chunk (3 x 96 = 288)
NCORES = 8

_CACHE = {}


def _build():
    nc = bacc.Bacc("TRN2", target_bir_lowering=False, debug=False,
                   num_devices=NCORES)
    x_d = nc.declare_dram_parameter("x", [N, T], F32, isOutput=False)
    te_d = nc.declare_dram_parameter("t_emb", [N, EMB], F32, isOutput=False)
    cc_d = nc.declare_dram_parameter("ccos", [T, F], F32, isOutput=False)
    cs_d = nc.declare_dram_parameter("csin", [T, F], F32, isOutput=False)
    wd_d = nc.declare_dram_parameter("wd0", [F, H], F32, isOutput=False)
    we_d = nc.declare_dram_parameter("we0", [H + EMB, H], F32, isOutput=False)
    w_d = nc.declare_dram_parameter("w", [H, 1], F32, isOutput=False)
    out_d = nc.declare_dram_parameter("out", [N, N], F32, isOutput=True)

    with tile.TileContext(nc) as tc, ExitStack() as ctx:
        const = ctx.enter_context(tc.tile_pool(name="const", bufs=1))
        ident = const.tile([P, P], F32)
        masks.make_identity(nc, ident[:])
        cc_sb = const.tile([KC, 3, F], F32)
        cs_sb = const.tile([KC, 3, F], F32)
        for c in range(3):
            nc.sync.dma_start(cc_sb[:, c, :], cc_d[c * KC:(c + 1) * KC, :])
            nc.sync.dma_start(cs_sb[:, c, :], cs_d[c * KC:(c + 1) * KC, :])
        wd_a = const.tile([P, H], F32)
        wd_b = const.tile([F - P, H], F32)
        nc.sync.dma_start(wd_a[:], wd_d[0:P, :])
        nc.sync.dma_start(wd_b[:], wd_d[P:F, :])
        we_sb = const.tile([H + EMB, H], F32)
        nc.sync.dma_start(we_sb[:], we_d[:])
        w_sb = const.tile([H, 1], F32)
        nc.sync.dma_start(w_sb[:], w_d[:])

        # persistent phase-1 buffers
        p1 = ctx.enter_context(tc.tile_pool(name="p1", bufs=1))
        re2 = p1.tile([P, NT, F], F32)
        im2 = p1.tile([P, NT, F], F32)
        mag = p1.tile([P, NT, F], F32)
        xn_all = p1.tile([P, NT, F], F32)
        te_all = p1.tile([P, NT, EMB], F32)
        ten_all = p1.tile([P, NT, EMB], F32)
        cat_all = p1.tile([P, NT, H + EMB], F32)
        hr_all = p1.tile([P, NT, H], F32)
        hc_all = p1.tile([P, NT, H], F32)
        hT_sb = p1.tile([H, N], F32)
        hTw_sb = p1.tile([H, N], F32)
        # [P, NT] stats
        st = ctx.enter_context(tc.tile_pool(name="stats", bufs=1))
        mx_s = st.tile([P, NT], F32)
        mn_s = st.tile([P, NT], F32)
        rd_s = st.tile([P, NT], F32)
        ssx_s = st.tile([P, NT], F32)
        rnx_s = st.tile([P, NT], F32)
        mxt_s = st.tile([P, NT], F32)
        mnt_s = st.tile([P, NT], F32)
        rdt_s = st.tile([P, NT], F32)
        sst_s = st.tile([P, NT], F32)
        rnt_s = st.tile([P, NT], F32)
        sums_s = st.tile([P, NT], F32)
        mean_s = st.tile([P, NT], F32)
        ssh_s = st.tile([P, NT], F32)
        rstd_s = st.tile([P, NT], F32)
        tmp_s = st.tile([P, NT], F32)

        # ---- loop A: x load, transpose, DFT matmuls, |X|^2 pieces ----
        with tc.tile_pool(name="la_sb", bufs=3) as la_sb, \
             tc.tile_pool(name="la_ps", bufs=2, space="PSUM") as la_ps:
            for t in range(NT):
                x_t = la_sb.tile([P, T], F32, tag="x")
                nc.sync.dma_start(x_t[:], x_d[t * P:(t + 1) * P, :])
                te_t = te_all[:, t, :]
                nc.sync.dma_start(te_t, te_d[t * P:(t + 1) * P, :])
                xT = la_sb.tile([KC, 3, P], F32, tag="xT")
                for c in range(3):
                    ps = la_ps.tile([KC, P], F32, tag="xT_ps")
                    nc.tensor.transpose(ps[:], x_t[:, c * KC:(c + 1) * KC], ident[:])
                    nc.any.tensor_copy(xT[:, c, :], ps[:])
                re_ps = la_ps.tile([P, F], F32, tag="re_ps")
                im_ps = la_ps.tile([P, F], F32, tag="im_ps")
                for c in range(3):
                    nc.tensor.matmul(re_ps[:], lhsT=xT[:, c, :], rhs=cc_sb[:, c, :],
                                     start=(c == 0), stop=(c == 2))
                for c in range(3):
                    nc.tensor.matmul(im_ps[:], lhsT=xT[:, c, :], rhs=cs_sb[:, c, :],
                                     start=(c == 0), stop=(c == 2))
                nc.scalar.square(re2[:, t, :], re_ps[:])
                nc.scalar.square(im2[:, t, :], im_ps[:])

        # ---- batched: mag, minmax, te minmax ----
        nc.vector.tensor_add(mag[:], re2[:], im2[:])
        nc.scalar.sqrt(mag[:], mag[:])
        nc.vector.tensor_reduce(mx_s[:], mag[:], axis=AX.X, op=OP.max)
        nc.vector.tensor_reduce(mn_s[:], mag[:], axis=AX.X, op=OP.min)
        nc.vector.scalar_tensor_tensor(tmp_s[:], mx_s[:], 1.0, mn_s[:], op0=OP.add, op1=OP.subtract)
        nc.vector.reciprocal(rd_s[:], tmp_s[:])
        nc.vector.tensor_reduce(mxt_s[:], te_all[:], axis=AX.X, op=OP.max)
        nc.vector.tensor_reduce(mnt_s[:], te_all[:], axis=AX.X, op=OP.min)
        nc.vector.scalar_tensor_tensor(tmp_s[:], mxt_s[:], 1.0, mnt_s[:], op0=OP.add, op1=OP.subtract)
        nc.vector.reciprocal(rdt_s[:], tmp_s[:])

        # ---- per-tile: xn = (mag-mn)*rd, l2 sums; te likewise ----
        with tc.tile_pool(name="lb_sb", bufs=2) as lb_sb:
            for t in range(NT):
                nc.vector.tensor_scalar(xn_all[:, t, :], mag[:, t, :],
                                        scalar1=mn_s[:, t:t + 1], scalar2=rd_s[:, t:t + 1],
                                        op0=OP.subtract, op1=OP.mult)
                scr = lb_sb.tile([P, F], F32, tag="scrF")
                nc.scalar.activation(scr[:], xn_all[:, t, :], AF.Square,
                                     accum_out=ssx_s[:, t:t + 1])
                nc.vector.tensor_scalar(ten_all[:, t, :], te_all[:, t, :],
                                        scalar1=mnt_s[:, t:t + 1], scalar2=rdt_s[:, t:t + 1],
                                        op0=OP.subtract, op1=OP.mult)
                scr2 = lb_sb.tile([P, EMB], F32, tag="scrE")
                nc.scalar.activation(scr2[:], ten_all[:, t, :], AF.Square,
                                     accum_out=sst_s[:, t:t + 1])
        nc.scalar.sqrt(ssx_s[:], ssx_s[:])
        nc.vector.reciprocal(rnx_s[:], ssx_s[:])
        nc.scalar.sqrt(sst_s[:], sst_s[:])
        nc.vector.reciprocal(rnt_s[:], sst_s[:])

        # ---- loop B1: q = xn @ Wd0 (via PE transpose), build cat ----
        with tc.tile_pool(name="b1_sb", bufs=3) as b1_sb, \
             tc.tile_pool(name="b1_ps", bufs=2, space="PSUM") as b1_ps:
            for t in range(NT):
                pa = b1_ps.tile([P, P], F32, tag="xnT_a_ps")
                nc.tensor.transpose(pa[:], xn_all[:, t, 0:P], ident[:])
                pb = b1_ps.tile([F - P, P], F32, tag="xnT_b_ps")
                nc.tensor.transpose(pb[:], xn_all[:, t, P:F], ident[:])
                xnT_a = b1_sb.tile([P, P], F32, tag="xnT_a")
                xnT_b = b1_sb.tile([F - P, P], F32, tag="xnT_b")
                nc.any.tensor_copy(xnT_a[:], pa[:])
                nc.any.tensor_copy(xnT_b[:], pb[:])
                q_ps = b1_ps.tile([P, H], F32, tag="q_ps")
                nc.tensor.matmul(q_ps[:], lhsT=xnT_a[:], rhs=wd_a[:], start=True, stop=False)
                nc.tensor.matmul(q_ps[:], lhsT=xnT_b[:], rhs=wd_b[:], start=False, stop=True)
                nc.scalar.activation(cat_all[:, t, 0:H], q_ps[:], AF.Copy,
                                     scale=rnx_s[:, t:t + 1])
                nc.vector.tensor_scalar_mul(cat_all[:, t, H:H + EMB], ten_all[:, t, :],
                                            rnt_s[:, t:t + 1])

        # ---- loop B2: h = relu(cat @ We0), rowsums ----
        with tc.tile_pool(name="b2_sb", bufs=3) as b2_sb, \
             tc.tile_pool(name="b2_ps", bufs=2, space="PSUM") as b2_ps:
            for t in range(NT):
                pc = b2_ps.tile([H + EMB, P], F32, tag="catT_ps")
                nc.tensor.transpose(pc[:], cat_all[:, t, :], ident[:])
                catT = b2_sb.tile([H + EMB, P], F32, tag="catT")
                nc.any.tensor_copy(catT[:], pc[:])
                h_ps = b2_ps.tile([P, H], F32, tag="h_ps")
                nc.tensor.matmul(h_ps[:], lhsT=catT[:], rhs=we_sb[:], start=True, stop=True)
                nc.scalar.activation(hr_all[:, t, :], h_ps[:], AF.Relu,
                                     accum_out=sums_s[:, t:t + 1])

        # ---- LN stats + h final + transposes ----
        nc.vector.tensor_scalar_mul(mean_s[:], sums_s[:], 1.0 / H)
        with tc.tile_pool(name="lc_sb", bufs=2) as lc_sb:
            for t in range(NT):
                nc.vector.tensor_scalar(hc_all[:, t, :], hr_all[:, t, :],
                                        scalar1=mean_s[:, t:t + 1], scalar2=None,
                                        op0=OP.subtract)
                scr = lc_sb.tile([P, H], F32, tag="scrH")
                nc.scalar.activation(scr[:], hc_all[:, t, :], AF.Square,
                                     accum_out=ssh_s[:, t:t + 1])
        nc.vector.tensor_scalar(ssh_s[:], ssh_s[:], scalar1=1.0 / H, scalar2=1e-8,
                                op0=OP.mult, op1=OP.add)
        nc.scalar.sqrt(ssh_s[:], ssh_s[:])
        nc.vector.reciprocal(rstd_s[:], ssh_s[:])
        with tc.tile_pool(name="lc2_sb", bufs=3) as lc2_sb, \
             tc.tile_pool(name="lc2_ps", bufs=2, space="PSUM") as lc2_ps:
            for t in range(NT):
                h_t = lc2_sb.tile([P, H], F32, tag="h_t")
                nc.vector.tensor_scalar_mul(h_t[:], hc_all[:, t, :], rstd_s[:, t:t + 1])
                hT_ps = lc2_ps.tile([H, P], F32, tag="hT_ps")
                nc.tensor.transpose(hT_ps[:], h_t[:], ident[:])
                nc.scalar.copy(hT_sb[:, t * P:(t + 1) * P], hT_ps[:])
                nc.vector.tensor_scalar_mul(hTw_sb[:, t * P:(t + 1) * P], hT_ps[:],
                                            w_sb[:, 0:1])

        # ---- phase 2: adjacency + top-k + normalize ----
        with tc.tile_pool(name="p2_sb", bufs=3) as p2_sb, \
             tc.tile_pool(name="p2_zap", bufs=2) as p2_zap, \
             tc.tile_pool(name="p2_sm", bufs=3) as p2_sm, \
             tc.tile_pool(name="p2_ps", bufs=2, space="PSUM") as p2_ps:
            for m in range(NT):
                adj_ps = p2_ps.tile([P, N], F32, tag="adj_ps")
                for n in range(4):
                    nc.tensor.matmul(adj_ps[:, n * 512:(n + 1) * 512],
                                     lhsT=hTw_sb[:, m * P:(m + 1) * P],
                                     rhs=hT_sb[:, n * 512:(n + 1) * 512],
                                     start=True, stop=True)
                adj_sb = p2_sb.tile([P, N], F32, tag="adj_sb")
                nc.scalar.activation(adj_sb[:], adj_ps[:], AF.Relu)
                mx16 = p2_sm.tile([P, 16], F32, tag="mx16")
                nc.vector.max(mx16[:, 0:8], adj_sb[:])
                zap = p2_zap.tile([P, N], F32, tag="zap")
                nc.vector.match_replace(zap[:], in_to_replace=mx16[:, 0:8],
                                        in_values=adj_sb[:], imm_value=0.0)
                nc.vector.max(mx16[:, 8:16], zap[:])
                den = p2_sm.tile([P, 1], F32, tag="den")
                nc.vector.tensor_reduce(den[:], mx16[:, 0:TOPK], axis=AX.X, op=OP.add)
                r = p2_sm.tile([P, 1], F32, tag="r")
                nc.vector.tensor_scalar_add(den[:], den[:], 1e-5)
                nc.vector.reciprocal(r[:], den[:])
                sel = p2_sb.tile([P, N], F32, tag="sel")
                nc.vector.scalar_tensor_tensor(sel[:], adj_sb[:], mx16[:, TOPK - 1:TOPK],
                                               adj_sb[:], op0=OP.is_ge, op1=OP.mult)
                outt = p2_sb.tile([P, N], F32, tag="outt")
                nc.scalar.activation(outt[:], sel[:], AF.Copy, scale=r[:, 0:1])
                nc.sync.dma_start(out_d[m * P:(m + 1) * P, :], outt[:])

    nc.compile()
    return nc


def _dft_mats():
    tt = np.arange(T)[:, None].astype(np.float64)
    kk = np.arange(F)[None, :].astype(np.float64)
    ang = 2.0 * np.pi * tt * kk / T
    s = 1.0 / np.sqrt(T)
    return (np.cos(ang) * s).astype(np.float32), (np.sin(ang) * s).astype(np.float32)


def kernel(x, t_emb, Wd0, We0, W):
    if "nc" not in _CACHE:
        _CACHE["nc"] = _build()
    nc = _CACHE["nc"]
    cc, cs = _dft_mats()
    base = {
        "ccos": cc, "csin": cs,
        "wd0": np.ascontiguousarray(Wd0, np.float32),
        "we0": np.ascontiguousarray(We0, np.float32),
        "w": np.ascontiguousarray(W, np.float32),
    }
    in_maps = [
        {**base,
         "x": np.ascontiguousarray(x[i], np.float32),
         "t_emb": np.ascontiguousarray(t_emb[i], np.float32)}
        for i in range(NCORES)
    ]
    res = run_bass_kernel_spmd(nc, in_maps, list(range(NCORES)))
    return np.stack([res.results[i]["out"] for i in range(NCORES)], axis=0)
